# revision 1
# baseline (speedup 1.0000x reference)
"""DeltaMPredictor Trainium2 kernel (8 NeuronCores, data-parallel over batch).

Pipeline per token (b, c):
    reg = thumb @ proj_w.T + proj_b            [2048] -> [512]
    y   = (reg - mean) * rstd                  per-camera LayerNorm (gamma
                                               folded into the SwiGLU weights)
    gate = y @ (w_gate*gamma).T (+ w_gate@beta via ACT bias)
    val  = y @ (w_val *gamma).T (+ w_val @beta via STT bias)
    h   = silu(gate) * val
    A   = reshape(h @ w_out.T, 6, 6); A -= A.T; clip frob to 3
    dM  = expm(A)   even/odd degree-5 Taylor + 1 squaring (4 bprods total):
                    B = As@As, B2 = B@B, E0 = I + B/2 + B2/24 + As@C2,
                    C2 = I + B/6 + B2/120, E = E0@E0

Sharding: batch B=16384 split 8 ways (2048 rows/core); weights replicated.
Per core the loop is camera-major (4 cameras x 4 tiles of 512 tokens).

Perf structure (from trace analysis of the previous version):
  - expm's broadcast TTs are the DVE/GPSIMD bottleneck (~3-4 ns/elem on HW);
    cutting 9 bprods -> 4 halves that work.
  - expm for tile t is ISSUED during tile t+1 (after mm2, before mm3) so the
    LN/y chain of the current tile always outranks expm backlog on the
    DVE/GPSIMD/ACT priority heaps; the PE stream never waits on expm.
  - outputs are DMA'd per tile on the scalar HWDGE queue (keeps the sync
    queue free for thumbnail loads).
  - a chain of dummy PE transposes warms the HAM clock gate during the
    initial DMA wait, so mm1 starts at 2.4 GHz.
"""

import os
import sys

sys.path.insert(0, "/opt/trn_rl_repo")

from contextlib import ExitStack

import numpy as np

import concourse.bacc as bacc
import concourse.bass as bass
import concourse.tile as tile
from concourse import mybir
from concourse.bass_utils import run_bass_kernel_spmd
from concourse.masks import make_identity

B, C, D_BB, D = 16384, 4, 2048, 512
N_CORES = 8
BLOC = B // N_CORES          # 2048 batch rows per core
TOK = 512                    # tokens per tile
NT = BLOC // TOK             # 4 tiles per camera
G = TOK // 128               # 4 token chunks of 128 per tile
KE = D_BB // 128             # 16 contraction chunks for mm1
KD = D // 128                # 4 contraction chunks for mm2/mm3
MAX_NORM = 3.0
LN_EPS = 1e-5
EXP_S = 1                    # squarings in expm (deg-5 even/odd Taylor)
WARM_N = 48                  # dummy PE transposes to warm the clock gate

F32 = mybir.dt.float32
F32R = mybir.dt.float32r
BF16 = mybir.dt.bfloat16
I32 = mybir.dt.int32
AL = mybir.AluOpType
AF = mybir.ActivationFunctionType
AX = mybir.AxisListType

# Priority boost for the tile-critical chain (LN stats -> rstd -> y -> yT
# copies -> silu -> h).  Priorities only break ties among READY ops, so the
# real protection is the engine split below; the boost still helps ties.
FG = 1_000_000

# Engine split rationale (from trace analysis): the list scheduler never
# idles an engine to wait for a higher-priority op that isn't ready yet, and
# its cost model underestimates the expm broadcast-TTs ~3-5x.  Any expm op
# placed on DVE/ACT therefore head-of-line-blocks the tile-critical LN/silu
# chain by ~0.5-1us at every entry point.  So: ACT and (mostly) DVE carry
# ONLY foreground work; GPSIMD carries the whole expm except the four big
# l-reduces, which stay on DVE but are split in half to shrink the blocking
# quantum to ~0.5us.

_BUILD_CACHE = {}
last_results = None          # test harness introspection
last_in_maps = None


def _bprod(nc, pool, left, right, tag, out_tile=None):
    """out = left @ right per (token, g): one broadcast multiply per g into
    TMP (i, j, l layout), all on GPSIMD (pure background engine), then the
    segmented l-reduce on DVE split in two halves so each foreground-
    blocking op stays ~0.5us."""
    TMP = pool.tile([128, G * 216], F32, tag="xTMP")
    for g in range(G):
        lv = (
            left[:, g * 36 : (g + 1) * 36]
            .rearrange("p (i l) -> p i l", i=6)
            .unsqueeze(2)
            .broadcast_to((128, 6, 6, 6))          # p i j l
        )
        rv = (
            right[:, g * 36 : (g + 1) * 36]
            .rearrange("p (l j) -> p l j", l=6)
            .unsqueeze(1)
            .broadcast_to((128, 6, 6, 6))          # p i l j
            .transpose([0, 1, 3, 2])               # p i j l
        )
        tmp_v = TMP[:, g * 216 : (g + 1) * 216].rearrange(
            "p (i j l) -> p i j l", i=6, j=6, l=6
        )
        nc.gpsimd.tensor_tensor(out=tmp_v, in0=lv, in1=rv, op=AL.mult)
    if out_tile is None:
        out_tile = pool.tile([128, G * 36], F32, tag=tag)
    hg = G // 2
    for hh in range(2):
        nc.vector.tensor_reduce(
            out=out_tile[:, hh * hg * 36 : (hh + 1) * hg * 36],
            in_=TMP[
                :, hh * hg * 216 : (hh + 1) * hg * 216
            ].rearrange("p (q l) -> p q l", l=6),
            axis=AX.X,
            op=AL.add,
        )
    return out_tile


def _diag_add_one(nc, t, consts):
    # stays on GPSIMD (background): an ACT op here would wait on the GPS-
    # produced input and head-of-line-block the foreground ACT stream.
    dv = t[:, :].rearrange("p (g a) -> p g a", g=G)[:, :, 0:36:7]
    ones = consts["cone"][:, 0:1].unsqueeze(2).broadcast_to((128, G, 6))
    nc.gpsimd.tensor_tensor(out=dv, in0=dv, in1=ones, op=AL.add)


def _cb(ct, n=G * 36):
    return ct[:, 0:1].broadcast_to((128, n))


def _expm_stage1(nc, pool, A0, consts):
    """Skew, frob-clip, B = As@As, B2 = B@B.  Returns tiles for stage 2.

    All elementwise work on GPSIMD via tensor_tensor against broadcast
    constant tiles (NEVER gpsimd.tensor_scalar -- its Q7 software path runs
    ~1.4us per op vs ~0.4-0.7us for tensor_tensor).
    """

    def v4(t):
        return t[:, :].rearrange("p (g i j) -> p g i j", g=G, i=6, j=6)

    S = pool.tile([128, G * 36], F32, tag="xS")
    nc.gpsimd.tensor_tensor(
        out=v4(S), in0=v4(A0), in1=v4(A0).transpose([0, 1, 3, 2]), op=AL.subtract
    )
    SQ = pool.tile([128, G * 36], F32, tag="xSQ")
    nc.gpsimd.tensor_tensor(out=SQ[:, :], in0=S[:, :], in1=S[:, :], op=AL.mult)
    ss = pool.tile([128, G], F32, tag="xss")
    nc.vector.tensor_reduce(
        out=ss[:, :],
        in_=SQ[:, :].rearrange("p (g a) -> p g a", g=G),
        axis=AX.X,
        op=AL.add,
    )
    # scs = min(MAX_NORM/frob, 1)/2^EXP_S, computed clamp-first so the
    # whole chain is DVE -> GPS with no downstream waits and no infinities:
    # scs = pow(max(ss, MN^2) / (MN/2^s)^2, -1/2)
    m2 = pool.tile([128, G], F32, tag="xm2")
    nc.vector.tensor_scalar(
        out=m2[:, :], in0=ss[:, :],
        scalar1=MAX_NORM * MAX_NORM,
        scalar2=(1 << EXP_S) * (1 << EXP_S) / (MAX_NORM * MAX_NORM),
        op0=AL.max, op1=AL.mult,
    )
    scs = pool.tile([128, G], F32, tag="xscs")
    nc.gpsimd.tensor_tensor(
        out=scs[:, :], in0=m2[:, :],
        in1=_cb(consts["neghalf"], G), op=AL.pow,
    )
    As = pool.tile([128, G * 36], F32, tag="xAs")
    for g in range(G):
        nc.gpsimd.tensor_tensor(
            out=As[:, g * 36 : (g + 1) * 36],
            in0=S[:, g * 36 : (g + 1) * 36],
            in1=scs[:, g : g + 1].broadcast_to((128, 36)),
            op=AL.mult,
        )
    Bm = _bprod(nc, pool, As, As, "xB")
    B2 = _bprod(nc, pool, Bm, Bm, "xB2")
    return {"As": As, "Bm": Bm, "B2": B2}


def _expm_stage2(nc, pool, st, out_slice, consts):
    """C2 = I + B/6 + B2/120, E0 = I + B/2 + B2/24 + As@C2, E = E0@E0."""
    As, Bm, B2 = st["As"], st["Bm"], st["B2"]
    # C2 = (B + B2/20)/6 + I
    w = pool.tile([128, G * 36], F32, tag="xw2")
    nc.gpsimd.tensor_tensor(
        out=w[:, :], in0=B2[:, :], in1=_cb(consts["c20"]), op=AL.mult
    )
    nc.gpsimd.tensor_tensor(out=w[:, :], in0=w[:, :], in1=Bm[:, :], op=AL.add)
    C2 = pool.tile([128, G * 36], F32, tag="xC2")
    nc.gpsimd.tensor_tensor(
        out=C2[:, :], in0=w[:, :], in1=_cb(consts["c6"]), op=AL.mult
    )
    _diag_add_one(nc, C2, consts)
    P = _bprod(nc, pool, As, C2, "xP")
    # E0 = (B + B2/12)/2 + P + I
    v = pool.tile([128, G * 36], F32, tag="xv")
    nc.gpsimd.tensor_tensor(
        out=v[:, :], in0=B2[:, :], in1=_cb(consts["c12"]), op=AL.mult
    )
    nc.gpsimd.tensor_tensor(out=v[:, :], in0=v[:, :], in1=Bm[:, :], op=AL.add)
    E0 = pool.tile([128, G * 36], F32, tag="xE0")
    nc.gpsimd.tensor_tensor(
        out=E0[:, :], in0=v[:, :], in1=_cb(consts["chalf"]), op=AL.mult
    )
    nc.gpsimd.tensor_tensor(out=E0[:, :], in0=E0[:, :], in1=P[:, :], op=AL.add)
    _diag_add_one(nc, E0, consts)
    _bprod(nc, pool, E0, E0, "xE", out_tile=out_slice)


def _build(emit_pb, emit_gb):
    nc = bacc.Bacc("TRN2", target_bir_lowering=False, debug=False)

    th = nc.dram_tensor("th", [C, NT, 128, KE * TOK], BF16, kind="ExternalInput")
    pwT = nc.dram_tensor("pwT", [128, KE * D], BF16, kind="ExternalInput")
    wgT = nc.dram_tensor("wgT", [C, 128, KD * D], F32R, kind="ExternalInput")
    wvT = nc.dram_tensor("wvT", [C, 128, KD * D], F32R, kind="ExternalInput")
    woT = nc.dram_tensor("woT", [C, 128, KD * 36], F32R, kind="ExternalInput")
    pb = bg = bv = None
    if emit_pb:
        pb = nc.dram_tensor("pb", [1, D], F32R, kind="ExternalInput")
    if emit_gb:
        # per-camera rank-1 LN-beta biases, f-major [128, KD] so column mf is
        # the per-partition bias for f-chunk mf.
        bg = nc.dram_tensor("bg", [C, 128, KD], F32, kind="ExternalInput")
        bv = nc.dram_tensor("bv", [C, 128, KD], F32, kind="ExternalInput")
    out = nc.dram_tensor("out", [128, C * NT * G * 36], F32, kind="ExternalOutput")

    with tile.TileContext(nc) as tc, ExitStack() as ctx:
        singles = ctx.enter_context(tc.tile_pool(name="singles", bufs=1))
        cam = ctx.enter_context(tc.tile_pool(name="cam", bufs=2))
        tkp = ctx.enter_context(tc.tile_pool(name="tkp", bufs=2))
        work = ctx.enter_context(tc.tile_pool(name="work", bufs=2))
        xw = ctx.enter_context(tc.tile_pool(name="xw", bufs=2))
        # PSUM budget (8 banks): transposes+a0 share 2, mm1-reg 3, gate/val 3.
        tr_ps = ctx.enter_context(tc.tile_pool(name="tr_ps", bufs=2, space="PSUM"))
        reg_psp = ctx.enter_context(tc.tile_pool(name="reg_ps", bufs=3, space="PSUM"))
        mm_ps = ctx.enter_context(tc.tile_pool(name="mm_ps", bufs=3, space="PSUM"))

        # memset cannot emit float32r directly; stage in f32 and cast-copy.
        identF = singles.tile([128, 128], F32)
        make_identity(nc, identF)
        identR = singles.tile([128, 128], F32R)
        nc.vector.tensor_copy(identR, identF)
        # PE clock-gate warmup during the initial DMA wait: a WAW chain of
        # transposes whose final result becomes the live transpose identity.
        warm_ps = tr_ps.tile([128, 128], F32R, tag="tr")
        for _ in range(WARM_N):
            nc.tensor.transpose(warm_ps, identR, identR)
        identW = singles.tile([128, 128], F32R)
        nc.scalar.copy(identW, warm_ps)

        # pw on the scalar HWDGE queue (idle at start) so it streams in
        # parallel with the first thumbnail tile on the sync queue; mm1(t0)
        # is then paced by aggregate HBM bandwidth, not queue serialization.
        pw_s = singles.tile([128, KE, D], BF16)
        pwsrc = pwT.ap().rearrange("p (k d) -> p k d", k=KE)
        for q in range(4):
            nc.scalar.dma_start(
                pw_s[:, q * 4 : (q + 1) * 4, :], pwsrc[:, q * 4 : (q + 1) * 4, :]
            )
        pb_s = None
        ones128 = None
        if emit_pb:
            onesF = singles.tile([1, 128], F32)
            nc.vector.memset(onesF, 1.0)
            ones128 = singles.tile([1, 128], F32R)
            nc.vector.tensor_copy(ones128, onesF)
            pb_s = singles.tile([1, D], F32R)
            nc.gpsimd.dma_start(pb_s, pb.ap())
        magic = singles.tile([128, 1], I32)
        nc.vector.memset(magic, 0x5F3759DF)
        consts = {}
        for nm, val in (
            ("neghalf", -0.5),
            ("clipm", MAX_NORM / (1 << EXP_S)),
            ("clipc", 1.0 / (1 << EXP_S)),
            ("c20", 1.0 / 20.0),
            ("c6", 1.0 / 6.0),
            ("c12", 1.0 / 12.0),
            ("chalf", 0.5),
            ("cone", 1.0),
        ):
            ct = singles.tile([128, 1], F32, name=f"c_{nm}")
            nc.vector.memset(ct, val)
            consts[nm] = ct

        # expm runs as a 2-stage pipeline, each stage one tile behind the
        # previous, so the serial GPSIMD chain of one expm (~18us) spans two
        # tile periods instead of gating one.
        q1 = None  # awaiting stage 1:  (A0, idx)
        q2 = None  # awaiting stage 2:  (state, idx)

        # ---- 3-deep software pipeline over tiles ----------------------
        # body tau:  mm1+LN(tau) | tr+mm2+silu/h(tau-1) | expm stages | 
        #            mm3+A0(tau-2)
        # Every PE op in a body depends only on scalar-chain results from a
        # FULL tile earlier (~23us of slack vs ~5us chain latency), so the
        # sim's optimistic latency model can no longer expose PE stalls.

        def emit_mm1_ln(c, ti):
            thsrc = th.ap()[c, ti].rearrange("p (k b) -> p k b", k=KE)
            qk = KE // 4
            thq = []
            for q in range(4):
                t_ = tkp.tile([128, qk, TOK], BF16, tag=f"tk{q}")
                nc.sync.dma_start(t_, thsrc[:, q * qk : (q + 1) * qk, :])
                thq.append(t_)
            y = work.tile([128, G, D], F32R, tag="y", bufs=2)
            mv = work.tile([128, G, 2], F32, tag="mv")
            Ds = [None] * G
            reg_banks = []
            for g in range(G):
                reg_ps = reg_psp.tile([128, D], F32, tag="reg")
                reg_banks.append(reg_ps)
                for k in range(KE):
                    nc.tensor.matmul(
                        reg_ps,
                        thq[k // qk][:, k % qk, g * 128 : (g + 1) * 128],
                        pw_s[:, k, :],
                        start=(k == 0),
                        stop=(k == KE - 1 and not emit_pb),
                    )
                if emit_pb:
                    nc.tensor.matmul(
                        reg_ps, ones128, pb_s, start=False, stop=True
                    )
                # v8 scheme: DVE stats+negmu, GPS rstd+nmr, ACT y pair.
                with tc.high_priority(offset=FG):
                    st = work.tile([128, 6], F32, tag="bst")
                    nc.vector.bn_stats(out=st[:, :], in_=reg_ps)
                    nc.vector.bn_aggr(out=mv[:, g, :], in_=st[:, :])
                    if g % 2 == 1:
                        p0 = g - 1
                        varv = mv[:, p0 : g + 1, 1:2].rearrange(
                            "p a b -> p (a b)"
                        )
                        meanv = mv[:, p0 : g + 1, 0:1].rearrange(
                            "p a b -> p (a b)"
                        )
                        negmu = work.tile([128, 2], F32, tag="lnng")
                        nc.vector.tensor_scalar(
                            out=negmu[:, :], in0=meanv, scalar1=-1.0,
                            scalar2=None, op0=AL.mult,
                        )
                        rstd = work.tile([128, 2], F32, tag="lnrs")
                        nc.gpsimd.tensor_tensor(
                            out=rstd[:, :], in0=varv,
                            in1=consts["neghalf"][:, 0:1]
                            .broadcast_to((128, 2)),
                            op=AL.pow,
                        )
                        nmr = work.tile([128, 2], F32, tag="lnnm")
                        nc.gpsimd.tensor_tensor(
                            out=nmr[:, :], in0=negmu[:, :],
                            in1=rstd[:, :], op=AL.mult,
                        )
                        for gg in (p0, g):
                            nc.scalar.activation(
                                y[:, gg, :], reg_banks[gg], AF.Identity,
                                bias=nmr[:, gg - p0 : gg - p0 + 1],
                                scale=rstd[:, gg - p0 : gg - p0 + 1],
                            )
                            Ds[gg] = identW
            return y, Ds

        def emit_tr_mm2(sA):
            y = sA["y"]
            yT = work.tile([128, KD, TOK], F32R, tag="yT", bufs=1)
            for kd in range(KD):
                tr = tr_ps.tile([128, TOK], F32R, tag="tr")
                for g in range(G):
                    nc.tensor.transpose(
                        tr[:, g * 128 : (g + 1) * 128],
                        y[:, g, kd * 128 : (kd + 1) * 128],
                        sA["D"][g],
                    )
                with tc.high_priority(offset=FG):
                    if kd % 2 == 0:
                        nc.scalar.copy(yT[:, kd, :], tr)
                    else:
                        nc.vector.tensor_copy(yT[:, kd, :], tr)
            h = work.tile([128, KD, TOK], F32R, tag="h")
            for mf in range(KD):
                g_ps = mm_ps.tile([128, TOK], F32, tag="mm")
                for kd in range(KD):
                    nc.tensor.matmul(
                        g_ps,
                        sA["wg"][:, kd, mf * 128 : (mf + 1) * 128],
                        yT[:, kd, :],
                        start=(kd == 0),
                        stop=(kd == KD - 1),
                    )
                v_ps = mm_ps.tile([128, TOK], F32, tag="mm")
                for kd in range(KD):
                    nc.tensor.matmul(
                        v_ps,
                        sA["wv"][:, kd, mf * 128 : (mf + 1) * 128],
                        yT[:, kd, :],
                        start=(kd == 0),
                        stop=(kd == KD - 1),
                    )
                sg = work.tile([128, TOK], F32, tag="sg", bufs=2)
                with tc.high_priority(offset=FG):
                    if emit_gb:
                        nc.scalar.activation(
                            sg, g_ps, AF.Silu, bias=sA["bg"][:, mf : mf + 1]
                        )
                        nc.vector.scalar_tensor_tensor(
                            out=h[:, mf, :], in0=v_ps,
                            scalar=sA["bv"][:, mf : mf + 1], in1=sg,
                            op0=AL.add, op1=AL.mult,
                        )
                    else:
                        nc.scalar.activation(sg, g_ps, AF.Silu)
                        nc.vector.tensor_tensor(
                            out=h[:, mf, :], in0=sg, in1=v_ps, op=AL.mult
                        )
            return h

        def emit_mm3(sB):
            a0_ps = tr_ps.tile([128, G, 36], F32R, tag="tr")
            a0T_ps = mm_ps.tile([36, TOK], F32, tag="mm", name="a0T")
            for kf in range(KD):
                nc.tensor.matmul(
                    a0T_ps,
                    sB["wo"][:, kf, :],
                    sB["h"][:, kf, :],
                    start=(kf == 0),
                    stop=(kf == KD - 1),
                )
            a0T_s = work.tile([36, TOK], F32R, tag="a0T")
            with tc.high_priority(offset=FG):
                nc.scalar.copy(a0T_s, a0T_ps)
            for g in range(G):
                nc.tensor.transpose(
                    a0_ps[:, g, :],
                    a0T_s[:, g * 128 : (g + 1) * 128],
                    identW[:36, :36],
                )
            A0 = xw.tile([128, G * 36], F32, tag="A0", bufs=3)
            with tc.high_priority(offset=FG):
                nc.scalar.copy(
                    A0, a0_ps[:, :, :].rearrange("p g a -> p (g a)")
                )
            return A0

        sA = sB = None       # tile tau-1 / tau-2 pipeline state
        q1 = q2 = None       # expm stage queues

        tiles = [(c, ti) for c in range(C) for ti in range(NT)]
        wg_s = wv_s = wo_s = bg_s = bv_s = None
        for step in range(len(tiles) + 2):
            if step < len(tiles):
                c, ti = tiles[step]
                if ti == 0:
                    wg_s = cam.tile([128, KD, D], F32R, tag="wg")
                    nc.gpsimd.dma_start(
                        wg_s, wgT.ap()[c].rearrange("p (k f) -> p k f", k=KD)
                    )
                    wv_s = cam.tile([128, KD, D], F32R, tag="wv")
                    nc.gpsimd.dma_start(
                        wv_s, wvT.ap()[c].rearrange("p (k f) -> p k f", k=KD)
                    )
                    wo_s = cam.tile([128, KD, 36], F32R, tag="wo")
                    nc.gpsimd.dma_start(
                        wo_s, woT.ap()[c].rearrange("p (k o) -> p k o", k=KD)
                    )
                    if emit_gb:
                        bg_s = cam.tile([128, KD], F32, tag="bg")
                        nc.gpsimd.dma_start(bg_s, bg.ap()[c])
                        bv_s = cam.tile([128, KD], F32, tag="bv")
                        nc.gpsimd.dma_start(bv_s, bv.ap()[c])
                y, Ds = emit_mm1_ln(c, ti)
            h = emit_tr_mm2(sA) if sA is not None else None
            if q2 is not None:
                st2, pidx = q2
                Eout = xw.tile([128, G * 36], F32, tag="Eout")
                _expm_stage2(nc, xw, st2, Eout, consts)
                nc.sync.dma_start(
                    out.ap()[:, pidx * G * 36 : (pidx + 1) * G * 36], Eout
                )
                q2 = None
            if q1 is not None:
                pA0, pidx = q1
                q2 = (_expm_stage1(nc, xw, pA0, consts), pidx)
                q1 = None
            if sB is not None:
                A0 = emit_mm3(sB)
                q1 = (A0, sB["idx"])
            sB = (
                {"h": h, "wo": sA["wo"], "idx": sA["idx"]}
                if sA is not None
                else None
            )
            if step < len(tiles):
                sA = {
                    "y": y, "D": Ds, "wg": wg_s, "wv": wv_s, "wo": wo_s,
                    "bg": bg_s, "bv": bv_s, "idx": c * NT + ti,
                }
            else:
                sA = None

        # drain the remaining expm stages
        for _ in range(2):
            if q2 is not None:
                st2, pidx = q2
                Eout = xw.tile([128, G * 36], F32, tag="Eout")
                _expm_stage2(nc, xw, st2, Eout, consts)
                nc.sync.dma_start(
                    out.ap()[:, pidx * G * 36 : (pidx + 1) * G * 36], Eout
                )
                q2 = None
            if q1 is not None:
                pA0, pidx = q1
                q2 = (_expm_stage1(nc, xw, pA0, consts), pidx)
                q1 = None

    nc.compile()
    return nc


def kernel(**inputs):
    global last_results, last_in_maps
    np_bf16 = mybir.dt.np(BF16)
    thumb = np.asarray(inputs["thumbnails"], dtype=np.float32)
    # [B, C, E] -> per-core [C, NT, 128, KE, TOK] so each tile lands with one
    # DMA whose per-partition data is contiguous (128 descriptors, not 2048).
    # bf16 halves the DMA bytes that pace both startup and the steady-state
    # thumbnail stream; matmul speed is unchanged (1 cyc/row either way).
    thB = thumb.reshape(N_CORES, NT, TOK, C, KE, 128)
    thB = np.ascontiguousarray(thB.transpose(0, 3, 1, 5, 4, 2)).astype(np_bf16)
    proj_w = np.asarray(inputs["proj_w"], dtype=np.float32)
    proj_b = np.asarray(inputs["proj_b"], dtype=np.float32)
    gamma = np.asarray(inputs["gamma"], dtype=np.float32)
    beta = np.asarray(inputs["beta"], dtype=np.float32)
    w_gate = np.asarray(inputs["w_gate"], dtype=np.float32)
    w_val = np.asarray(inputs["w_val"], dtype=np.float32)
    w_out = np.asarray(inputs["w_out"], dtype=np.float32)

    # host-side weight prep: fold gamma into the SwiGLU weights, beta into
    # rank-1 biases, pre-transpose everything for the PE's lhsT convention.
    def blockT(w):
        # [out, in] -> in-major [128, k, out]: lhsT chunks contiguous/partition
        o, i = w.shape
        return np.ascontiguousarray(
            w.T.reshape(i // 128, 128, o).transpose(1, 0, 2)
        ).reshape(128, i // 128 * o)

    pwT = blockT(proj_w).astype(np_bf16)                        # [128, KE*D]
    wgT = np.stack([blockT(w_gate[c] * gamma[c][None, :]) for c in range(C)])
    wvT = np.stack([blockT(w_val[c] * gamma[c][None, :]) for c in range(C)])
    woT = np.stack([blockT(w_out[c]) for c in range(C)])        # [C,128,KD*36]
    bg = np.einsum("cfd,cd->cf", w_gate, beta).astype(np.float32)
    bv = np.einsum("cfd,cd->cf", w_val, beta).astype(np.float32)

    emit_pb = bool(np.any(proj_b))
    emit_gb = bool(np.any(bg) or np.any(bv))

    key = (emit_pb, emit_gb)
    if key not in _BUILD_CACHE:
        _BUILD_CACHE[key] = _build(emit_pb, emit_gb)
    nc = _BUILD_CACHE[key]

    shared = {"pwT": pwT, "wgT": wgT, "wvT": wvT, "woT": woT}
    if emit_pb:
        shared["pb"] = proj_b.reshape(1, D)
    if emit_gb:
        # f-major [C, 128, KD]: bias for f = mf*128 + p lands at [p, mf]
        shared["bg"] = np.ascontiguousarray(
            bg.reshape(C, KD, 128).transpose(0, 2, 1)
        )
        shared["bv"] = np.ascontiguousarray(
            bv.reshape(C, KD, 128).transpose(0, 2, 1)
        )
    in_maps = []
    for i in range(N_CORES):
        m = dict(shared)
        m["th"] = thB[i].reshape(C, NT, 128, KE * TOK)
        in_maps.append(m)

    last_in_maps = in_maps
    trace = bool(int(os.environ.get("KERNEL_TRACE", "0")))
    try:
        last_results = run_bass_kernel_spmd(
            nc, in_maps, core_ids=list(range(N_CORES)), trace=trace
        )
    except ModuleNotFoundError:
        # tracing requested (e.g. BASS_TRACE in env) but the axon NTFF hook
        # module is absent in this image -- rerun without tracing.
        os.environ["BASS_NEVER_TRACE"] = "1"
        last_results = run_bass_kernel_spmd(
            nc, in_maps, core_ids=list(range(N_CORES)), trace=False
        )
    parts = []
    for r in last_results.results:
        o = r["out"].reshape(128, C, NT, G, 36)
        parts.append(o.transpose(1, 2, 3, 0, 4).reshape(C, BLOC, 36))
    full = np.concatenate(parts, axis=1)                        # [C, B, 36]
    return full.reshape(C, B, 6, 6)



# revision 2
# speedup vs baseline: 1.1128x; 1.1128x over previous
"""DeltaMPredictor Trainium2 kernel (8 NeuronCores, data-parallel over batch).

Pipeline per token (b, c):
    reg = thumb @ proj_w.T + proj_b            [2048] -> [512]
    y   = (reg - mean) * rstd                  per-camera LayerNorm (gamma
                                               folded into the SwiGLU weights)
    gate = y @ (w_gate*gamma).T (+ w_gate@beta via ACT bias)
    val  = y @ (w_val *gamma).T (+ w_val @beta via STT bias)
    h   = silu(gate) * val
    A   = reshape(h @ w_out.T, 6, 6); A -= A.T; clip frob to 3
    dM  = expm(A)   even/odd degree-5 Taylor + 1 squaring (4 bprods total):
                    B = As@As, B2 = B@B, E0 = I + B/2 + B2/24 + As@C2,
                    C2 = I + B/6 + B2/120, E = E0@E0

Sharding: batch B=16384 split 8 ways (2048 rows/core); weights replicated.
Per core the loop is camera-major (4 cameras x 4 tiles of 512 tokens).

Perf structure (v2, from trace analysis of the 622us baseline):
  - the 622us baseline lost ~107us to per-tile mm1 stalls waiting on
    thumbnail DMA (tkp bufs=2 releases the buffer too late) plus ~90us of
    HAM clock-gate penalty caused by those >3.4us PE idle gaps, ~43us to
    the expm drain tail, and ~11us to camera-boundary weight-DMA waits.
  - fixes: tkp bufs=3 (DMA issues a full extra tile early); output DMAs
    moved off the sync queue (they head-of-line-blocked thumbnail
    prefetch behind expm completion) onto the gpsimd SWDGE queue; camera
    weights moved from the gpsimd queue to the now-clean sync HWDGE
    queue; expm stage1 runs in the same step as mm3 so the drain tail is
    a single stage2 (~10us, was 3 stages / 43us).
  - engine balance: GPSIMD carries only background expm work (bprod
    multiplies, skew/scale, diag adds, scs pow).  The Taylor combines
    run as fused scalar_tensor_tensor/tensor_scalar ops on DVE.  The LN
    rstd chain moved from GPSIMD pow to DVE reciprocal + ACT sqrt, so no
    foreground op can queue behind a multi-us GPSIMD expm op.
  - y/yT/h and all mm2/mm3 weights are bf16: wg-stationary loads get FWL
    (f32r disables it), the y transposes run 1.0 cyc/row (f32r pays
    1.5x via the identity operand, which prices PE transpose cost), and
    the PSUM->SBUF yT copies run in DVE 2x mode.  Measured warm issue
    rates: bf16 N=512 matmul 216ns, f32r 233ns.
  - a chain of dummy PE transposes warms the HAM clock gate during the
    initial DMA wait, so mm1 starts at 2.4 GHz.
"""

import os
import sys

sys.path.insert(0, "/opt/trn_rl_repo")

from contextlib import ExitStack

import numpy as np

import concourse.bacc as bacc
import concourse.bass as bass
import concourse.tile as tile
from concourse import mybir
from concourse.bass_utils import run_bass_kernel_spmd
from concourse.masks import make_identity

B, C, D_BB, D = 16384, 4, 2048, 512
N_CORES = 8
BLOC = B // N_CORES          # 2048 batch rows per core
TOK = 512                    # tokens per tile
NT = BLOC // TOK             # 4 tiles per camera
G = TOK // 128               # 4 token chunks of 128 per tile
KE = D_BB // 128             # 16 contraction chunks for mm1
KD = D // 128                # 4 contraction chunks for mm2/mm3
MAX_NORM = 3.0
LN_EPS = 1e-5
EXP_S = 1                    # squarings in expm (deg-5 even/odd Taylor)
WARM_N = 48                  # dummy PE transposes to warm the clock gate

F32 = mybir.dt.float32
F32R = mybir.dt.float32r
BF16 = mybir.dt.bfloat16
I32 = mybir.dt.int32
AL = mybir.AluOpType
AF = mybir.ActivationFunctionType
AX = mybir.AxisListType

# Priority boost for the tile-critical chain (LN stats -> rstd -> y -> yT
# copies -> silu -> h).  Priorities only break ties among READY ops, so the
# real protection is the engine split; the boost still helps ties.
FG = 1_000_000

_BUILD_CACHE = {}
last_results = None          # test harness introspection
last_in_maps = None


def _bprod(nc, pool, left, right, tag, out_tile=None):
    """out = left @ right per (token, g): one broadcast multiply per g into
    TMP (i, j, l layout), all on GPSIMD (pure background engine), then the
    segmented l-reduce on DVE split in two halves so each foreground-
    blocking op stays ~0.5us."""
    TMP = pool.tile([128, G * 216], F32, tag="xTMP")
    for g in range(G):
        lv = (
            left[:, g * 36 : (g + 1) * 36]
            .rearrange("p (i l) -> p i l", i=6)
            .unsqueeze(2)
            .broadcast_to((128, 6, 6, 6))          # p i j l
        )
        rv = (
            right[:, g * 36 : (g + 1) * 36]
            .rearrange("p (l j) -> p l j", l=6)
            .unsqueeze(1)
            .broadcast_to((128, 6, 6, 6))          # p i l j
            .transpose([0, 1, 3, 2])               # p i j l
        )
        tmp_v = TMP[:, g * 216 : (g + 1) * 216].rearrange(
            "p (i j l) -> p i j l", i=6, j=6, l=6
        )
        nc.gpsimd.tensor_tensor(out=tmp_v, in0=lv, in1=rv, op=AL.mult)
    if out_tile is None:
        out_tile = pool.tile([128, G * 36], F32, tag=tag)
    hg = G // 2
    for hh in range(2):
        nc.vector.tensor_reduce(
            out=out_tile[:, hh * hg * 36 : (hh + 1) * hg * 36],
            in_=TMP[
                :, hh * hg * 216 : (hh + 1) * hg * 216
            ].rearrange("p (q l) -> p q l", l=6),
            axis=AX.X,
            op=AL.add,
        )
    return out_tile


def _diag_add_one(nc, t, consts):
    # stays on GPSIMD (background): an ACT op here would wait on the GPS/
    # DVE-produced input and head-of-line-block the foreground ACT stream.
    dv = t[:, :].rearrange("p (g a) -> p g a", g=G)[:, :, 0:36:7]
    ones = consts["cone"][:, 0:1].unsqueeze(2).broadcast_to((128, G, 6))
    nc.gpsimd.tensor_tensor(out=dv, in0=dv, in1=ones, op=AL.add)


def _cb(ct, n=G * 36):
    return ct[:, 0:1].broadcast_to((128, n))


def _expm_stage1(nc, pool, A0, consts):
    """Skew, frob-clip, B = As@As, B2 = B@B.  Returns tiles for stage 2.

    Elementwise work on GPSIMD via tensor_tensor against broadcast constant
    tiles (NEVER gpsimd.tensor_scalar -- its Q7 software path runs ~1.4us
    per op vs ~0.4-0.7us for tensor_tensor).
    """

    def v4(t):
        return t[:, :].rearrange("p (g i j) -> p g i j", g=G, i=6, j=6)

    S = pool.tile([128, G * 36], F32, tag="xS")
    nc.gpsimd.tensor_tensor(
        out=v4(S), in0=v4(A0), in1=v4(A0).transpose([0, 1, 3, 2]), op=AL.subtract
    )
    SQ = pool.tile([128, G * 36], F32, tag="xSQ")
    nc.gpsimd.tensor_tensor(out=SQ[:, :], in0=S[:, :], in1=S[:, :], op=AL.mult)
    ss = pool.tile([128, G], F32, tag="xss")
    nc.vector.tensor_reduce(
        out=ss[:, :],
        in_=SQ[:, :].rearrange("p (g a) -> p g a", g=G),
        axis=AX.X,
        op=AL.add,
    )
    # scs = min(MAX_NORM/frob, 1)/2^EXP_S, computed clamp-first so the
    # whole chain is DVE -> GPS with no downstream waits and no infinities:
    # scs = pow(max(ss, MN^2) / (MN/2^s)^2, -1/2)
    m2 = pool.tile([128, G], F32, tag="xm2")
    nc.vector.tensor_scalar(
        out=m2[:, :], in0=ss[:, :],
        scalar1=MAX_NORM * MAX_NORM,
        scalar2=(1 << EXP_S) * (1 << EXP_S) / (MAX_NORM * MAX_NORM),
        op0=AL.max, op1=AL.mult,
    )
    scs = pool.tile([128, G], F32, tag="xscs")
    nc.gpsimd.tensor_tensor(
        out=scs[:, :], in0=m2[:, :],
        in1=_cb(consts["neghalf"], G), op=AL.pow,
    )
    As = pool.tile([128, G * 36], F32, tag="xAs")
    nc.gpsimd.tensor_tensor(
        out=As[:, :].rearrange("p (g a) -> p g a", g=G),
        in0=S[:, :].rearrange("p (g a) -> p g a", g=G),
        in1=scs[:, :].unsqueeze(2).broadcast_to((128, G, 36)),
        op=AL.mult,
    )
    Bm = _bprod(nc, pool, As, As, "xB")
    B2 = _bprod(nc, pool, Bm, Bm, "xB2")
    return {"As": As, "Bm": Bm, "B2": B2}


def _expm_stage2(nc, pool, st, out_slice, consts):
    """C2 = I + B/6 + B2/120, E0 = I + B/2 + B2/24 + As@C2, E = E0@E0.

    Taylor combines run as fused DVE ops (scalar_tensor_tensor /
    tensor_scalar); only the bprod multiplies and diag adds touch GPSIMD.
    """
    As, Bm, B2 = st["As"], st["Bm"], st["B2"]
    # C2 = (B2/20 + B)/6 + I
    w = pool.tile([128, G * 36], F32, tag="xw2")
    nc.vector.scalar_tensor_tensor(
        out=w[:, :], in0=B2[:, :], scalar=1.0 / 20.0, in1=Bm[:, :],
        op0=AL.mult, op1=AL.add,
    )
    C2 = pool.tile([128, G * 36], F32, tag="xC2")
    nc.vector.tensor_scalar(
        out=C2[:, :], in0=w[:, :], scalar1=1.0 / 6.0, scalar2=None,
        op0=AL.mult,
    )
    _diag_add_one(nc, C2, consts)
    P = _bprod(nc, pool, As, C2, "xP")
    # E0 = (B2/12 + B)/2 + P + I
    E0 = pool.tile([128, G * 36], F32, tag="xE0")
    v = pool.tile([128, G * 36], F32, tag="xv")
    nc.vector.scalar_tensor_tensor(
        out=v[:, :], in0=B2[:, :], scalar=1.0 / 12.0, in1=Bm[:, :],
        op0=AL.mult, op1=AL.add,
    )
    nc.vector.scalar_tensor_tensor(
        out=E0[:, :], in0=v[:, :], scalar=0.5, in1=P[:, :],
        op0=AL.mult, op1=AL.add,
    )
    _diag_add_one(nc, E0, consts)
    _bprod(nc, pool, E0, E0, "xE", out_tile=out_slice)


def _build(emit_pb, emit_gb):
    nc = bacc.Bacc("TRN2", target_bir_lowering=False, debug=False)

    th = nc.dram_tensor("th", [C, NT, 128, KE * TOK], BF16, kind="ExternalInput")
    pwT = nc.dram_tensor("pwT", [128, KE * D], BF16, kind="ExternalInput")
    wgT = nc.dram_tensor("wgT", [C, 128, KD * D], BF16, kind="ExternalInput")
    wvT = nc.dram_tensor("wvT", [C, 128, KD * D], BF16, kind="ExternalInput")
    woT = nc.dram_tensor("woT", [C, 128, KD * 36], BF16, kind="ExternalInput")
    pb = bg = bv = None
    if emit_pb:
        pb = nc.dram_tensor("pb", [1, D], F32R, kind="ExternalInput")
    if emit_gb:
        # per-camera rank-1 LN-beta biases, f-major [128, KD] so column mf is
        # the per-partition bias for f-chunk mf.
        bg = nc.dram_tensor("bg", [C, 128, KD], F32, kind="ExternalInput")
        bv = nc.dram_tensor("bv", [C, 128, KD], F32, kind="ExternalInput")
    out = nc.dram_tensor("out", [128, C * NT * G * 36], F32, kind="ExternalOutput")

    with tile.TileContext(nc) as tc, ExitStack() as ctx:
        singles = ctx.enter_context(tc.tile_pool(name="singles", bufs=1))
        cam = ctx.enter_context(tc.tile_pool(name="cam", bufs=2))
        tkp = ctx.enter_context(tc.tile_pool(name="tkp", bufs=3))
        work = ctx.enter_context(tc.tile_pool(name="work", bufs=2))
        xw = ctx.enter_context(tc.tile_pool(name="xw", bufs=2))
        # PSUM budget (8 banks): transposes+a0 share 2, mm1-reg 3, gate/val 3.
        tr_ps = ctx.enter_context(tc.tile_pool(name="tr_ps", bufs=2, space="PSUM"))
        reg_psp = ctx.enter_context(tc.tile_pool(name="reg_ps", bufs=3, space="PSUM"))
        mm_ps = ctx.enter_context(tc.tile_pool(name="mm_ps", bufs=3, space="PSUM"))

        # memset cannot emit float32r directly; stage in f32 and cast-copy.
        identF = singles.tile([128, 128], F32)
        make_identity(nc, identF)
        identR = singles.tile([128, 128], F32R)
        nc.vector.tensor_copy(identR, identF)
        # PE clock-gate warmup during the initial DMA wait: a WAW chain of
        # transposes whose final result becomes the live transpose identity.
        warm_ps = tr_ps.tile([128, 128], F32R, tag="tr")
        for _ in range(WARM_N):
            nc.tensor.transpose(warm_ps, identR, identR)
        identW = singles.tile([128, 128], F32R)
        nc.scalar.copy(identW, warm_ps)
        # bf16 identity for the y transposes (bf16 transposes run 1 cyc/row;
        # the f32r identity operand would price them at 1.5).
        identB = singles.tile([128, 128], BF16)
        nc.vector.tensor_copy(identB, identF)

        # pw on the scalar HWDGE queue (idle at start) so it streams in
        # parallel with the first thumbnail tile on the sync queue; mm1(t0)
        # is then paced by aggregate HBM bandwidth, not queue serialization.
        pw_s = singles.tile([128, KE, D], BF16)
        pwsrc = pwT.ap().rearrange("p (k d) -> p k d", k=KE)
        for q in range(4):
            nc.scalar.dma_start(
                pw_s[:, q * 4 : (q + 1) * 4, :], pwsrc[:, q * 4 : (q + 1) * 4, :]
            )
        pb_s = None
        ones128 = None
        if emit_pb:
            onesF = singles.tile([1, 128], F32)
            nc.vector.memset(onesF, 1.0)
            ones128 = singles.tile([1, 128], F32R)
            nc.vector.tensor_copy(ones128, onesF)
            pb_s = singles.tile([1, D], F32R)
            nc.gpsimd.dma_start(pb_s, pb.ap())
        consts = {}
        for nm, val in (
            ("neghalf", -0.5),
            ("cone", 1.0),
        ):
            ct = singles.tile([128, 1], F32, name=f"c_{nm}")
            nc.vector.memset(ct, val)
            consts[nm] = ct

        # ---- software pipeline over tiles ------------------------------
        # step tau:  expm-stage2(tau-3)+out-DMA | mm3(tau-2)+expm-stage1 |
        #            mm1+LN(tau) | tr+mm2+silu/h(tau-1)
        # Every PE op in a step depends only on results from a FULL tile
        # earlier, and expm stage1 follows its own mm3 within the step, so
        # the drain after the last PE op is a single stage2 (~10us).

        def emit_mm1_ln(c, ti):
            thsrc = th.ap()[c, ti].rearrange("p (k b) -> p k b", k=KE)
            qk = KE // 4
            thq = []
            for q in range(4):
                t_ = tkp.tile([128, qk, TOK], BF16, tag=f"tk{q}")
                nc.sync.dma_start(t_, thsrc[:, q * qk : (q + 1) * qk, :])
                thq.append(t_)
            y = work.tile([128, G, D], BF16, tag="y", bufs=2)
            mv = work.tile([128, G, 2], F32, tag="mv")
            Ds = [None] * G
            reg_banks = []
            for g in range(G):
                reg_ps = reg_psp.tile([128, D], F32, tag="reg")
                reg_banks.append(reg_ps)
                for k in range(KE):
                    nc.tensor.matmul(
                        reg_ps,
                        thq[k // qk][:, k % qk, g * 128 : (g + 1) * 128],
                        pw_s[:, k, :],
                        start=(k == 0),
                        stop=(k == KE - 1 and not emit_pb),
                    )
                if emit_pb:
                    nc.tensor.matmul(
                        reg_ps, ones128, pb_s, start=False, stop=True
                    )
                # DVE stats; rstd via DVE reciprocal + ACT sqrt (no GPSIMD
                # in the foreground chain); ACT y pair.
                with tc.high_priority(offset=FG):
                    st = work.tile([128, 6], F32, tag="bst")
                    nc.vector.bn_stats(out=st[:, :], in_=reg_ps)
                    nc.vector.bn_aggr(out=mv[:, g, :], in_=st[:, :])
                    if g % 2 == 1:
                        p0 = g - 1
                        varv = mv[:, p0 : g + 1, 1:2].rearrange(
                            "p a b -> p (a b)"
                        )
                        meanv = mv[:, p0 : g + 1, 0:1].rearrange(
                            "p a b -> p (a b)"
                        )
                        iv = work.tile([128, 2], F32, tag="lniv")
                        nc.vector.tensor_scalar(
                            out=iv[:, :], in0=varv, scalar1=LN_EPS,
                            scalar2=None, op0=AL.add,
                        )
                        rv = work.tile([128, 2], F32, tag="lnrv")
                        nc.vector.reciprocal(rv[:, :], iv[:, :])
                        rstd = work.tile([128, 2], F32, tag="lnrs")
                        nc.scalar.sqrt(rstd[:, :], rv[:, :])
                        nmr = work.tile([128, 2], F32, tag="lnnm")
                        nc.vector.scalar_tensor_tensor(
                            out=nmr[:, :], in0=meanv, scalar=-1.0,
                            in1=rstd[:, :], op0=AL.mult, op1=AL.mult,
                        )
                        for gg in (p0, g):
                            nc.scalar.activation(
                                y[:, gg, :], reg_banks[gg], AF.Identity,
                                bias=nmr[:, gg - p0 : gg - p0 + 1],
                                scale=rstd[:, gg - p0 : gg - p0 + 1],
                            )
                            Ds[gg] = identB
            return y, Ds

        def emit_tr_mm2(sA):
            y = sA["y"]
            yT = work.tile([128, KD, TOK], BF16, tag="yT", bufs=1)
            for kd in range(KD):
                tr = tr_ps.tile([128, TOK], BF16, tag="tr")
                for g in range(G):
                    nc.tensor.transpose(
                        tr[:, g * 128 : (g + 1) * 128],
                        y[:, g, kd * 128 : (kd + 1) * 128],
                        sA["D"][g],
                    )
                with tc.high_priority(offset=FG):
                    if kd % 2 == 0:
                        nc.scalar.copy(yT[:, kd, :], tr)
                    else:
                        nc.vector.tensor_copy(yT[:, kd, :], tr)
            h = work.tile([128, KD, TOK], BF16, tag="h")
            for mf in range(KD):
                g_ps = mm_ps.tile([128, TOK], F32, tag="mm")
                for kd in range(KD):
                    nc.tensor.matmul(
                        g_ps,
                        sA["wg"][:, kd, mf * 128 : (mf + 1) * 128],
                        yT[:, kd, :],
                        start=(kd == 0),
                        stop=(kd == KD - 1),
                    )
                v_ps = mm_ps.tile([128, TOK], F32, tag="mm")
                for kd in range(KD):
                    nc.tensor.matmul(
                        v_ps,
                        sA["wv"][:, kd, mf * 128 : (mf + 1) * 128],
                        yT[:, kd, :],
                        start=(kd == 0),
                        stop=(kd == KD - 1),
                    )
                sg = work.tile([128, TOK], F32, tag="sg", bufs=2)
                with tc.high_priority(offset=FG):
                    if emit_gb:
                        nc.scalar.activation(
                            sg, g_ps, AF.Silu, bias=sA["bg"][:, mf : mf + 1]
                        )
                        nc.vector.scalar_tensor_tensor(
                            out=h[:, mf, :], in0=v_ps,
                            scalar=sA["bv"][:, mf : mf + 1], in1=sg,
                            op0=AL.add, op1=AL.mult,
                        )
                    else:
                        nc.scalar.activation(sg, g_ps, AF.Silu)
                        nc.vector.tensor_tensor(
                            out=h[:, mf, :], in0=sg, in1=v_ps, op=AL.mult
                        )
            return h

        def emit_mm3(sB):
            a0_ps = tr_ps.tile([128, G, 36], F32R, tag="tr")
            a0T_ps = mm_ps.tile([36, TOK], F32, tag="mm", name="a0T")
            for kf in range(KD):
                nc.tensor.matmul(
                    a0T_ps,
                    sB["wo"][:, kf, :],
                    sB["h"][:, kf, :],
                    start=(kf == 0),
                    stop=(kf == KD - 1),
                )
            a0T_s = work.tile([36, TOK], F32R, tag="a0T")
            with tc.high_priority(offset=FG):
                nc.scalar.copy(a0T_s, a0T_ps)
            for g in range(G):
                nc.tensor.transpose(
                    a0_ps[:, g, :],
                    a0T_s[:, g * 128 : (g + 1) * 128],
                    identW[:36, :36],
                )
            A0 = xw.tile([128, G * 36], F32, tag="A0", bufs=3)
            with tc.high_priority(offset=FG):
                nc.scalar.copy(
                    A0, a0_ps[:, :, :].rearrange("p g a -> p (g a)")
                )
            return A0

        sA = sB = None       # tile tau-1 / tau-2 pipeline state
        q2 = None            # expm awaiting stage 2:  (state, idx)

        tiles = [(c, ti) for c in range(C) for ti in range(NT)]
        wg_s = wv_s = wo_s = bg_s = bv_s = None
        for step in range(len(tiles) + 3):
            # stage 2 of the tile whose stage 1 ran last step; the output
            # DMA rides the gpsimd SWDGE queue (the sync queue must stay
            # free for thumbnail prefetch).
            pending = None
            if q2 is not None:
                st2, pidx = q2
                Eout = xw.tile([128, G * 36], F32, tag="Eout")
                _expm_stage2(nc, xw, st2, Eout, consts)
                pending = (pidx, Eout)
                q2 = None
            # mm3 + expm stage 1 in the same step: the drain after the
            # last PE op is a single stage 2.
            if sB is not None:
                A0 = emit_mm3(sB)
                q2 = (_expm_stage1(nc, xw, A0, consts), sB["idx"])
            if pending is not None:
                pidx, Eout = pending
                nc.gpsimd.dma_start(
                    out.ap()[:, pidx * G * 36 : (pidx + 1) * G * 36], Eout
                )
            if step < len(tiles):
                c, ti = tiles[step]
                if ti == 0:
                    # camera weights on the sync HWDGE queue: they are
                    # requested ~4 tiles early (cam bufs=2), so they never
                    # delay a thumbnail DMA behind them.
                    wg_s = cam.tile([128, KD, D], BF16, tag="wg")
                    nc.sync.dma_start(
                        wg_s, wgT.ap()[c].rearrange("p (k f) -> p k f", k=KD)
                    )
                    wv_s = cam.tile([128, KD, D], BF16, tag="wv")
                    nc.sync.dma_start(
                        wv_s, wvT.ap()[c].rearrange("p (k f) -> p k f", k=KD)
                    )
                    wo_s = cam.tile([128, KD, 36], BF16, tag="wo")
                    nc.sync.dma_start(
                        wo_s, woT.ap()[c].rearrange("p (k o) -> p k o", k=KD)
                    )
                    if emit_gb:
                        bg_s = cam.tile([128, KD], F32, tag="bg")
                        nc.sync.dma_start(bg_s, bg.ap()[c])
                        bv_s = cam.tile([128, KD], F32, tag="bv")
                        nc.sync.dma_start(bv_s, bv.ap()[c])
                y, Ds = emit_mm1_ln(c, ti)
            h = emit_tr_mm2(sA) if sA is not None else None
            sB = (
                {"h": h, "wo": sA["wo"], "idx": sA["idx"]}
                if sA is not None
                else None
            )
            if step < len(tiles):
                sA = {
                    "y": y, "D": Ds, "wg": wg_s, "wv": wv_s, "wo": wo_s,
                    "bg": bg_s, "bv": bv_s, "idx": c * NT + ti,
                }
            else:
                sA = None

    nc.compile()
    return nc


def kernel(**inputs):
    global last_results, last_in_maps
    np_bf16 = mybir.dt.np(BF16)
    thumb = np.asarray(inputs["thumbnails"], dtype=np.float32)
    # [B, C, E] -> per-core [C, NT, 128, KE, TOK] so each tile lands with one
    # DMA whose per-partition data is contiguous (128 descriptors, not 2048).
    # bf16 halves the DMA bytes that pace both startup and the steady-state
    # thumbnail stream; matmul speed is unchanged (1 cyc/row either way).
    thB = thumb.reshape(N_CORES, NT, TOK, C, KE, 128)
    thB = np.ascontiguousarray(thB.transpose(0, 3, 1, 5, 4, 2)).astype(np_bf16)
    proj_w = np.asarray(inputs["proj_w"], dtype=np.float32)
    proj_b = np.asarray(inputs["proj_b"], dtype=np.float32)
    gamma = np.asarray(inputs["gamma"], dtype=np.float32)
    beta = np.asarray(inputs["beta"], dtype=np.float32)
    w_gate = np.asarray(inputs["w_gate"], dtype=np.float32)
    w_val = np.asarray(inputs["w_val"], dtype=np.float32)
    w_out = np.asarray(inputs["w_out"], dtype=np.float32)

    # host-side weight prep: fold gamma into the SwiGLU weights, beta into
    # rank-1 biases, pre-transpose everything for the PE's lhsT convention.
    def blockT(w):
        # [out, in] -> in-major [128, k, out]: lhsT chunks contiguous/partition
        o, i = w.shape
        return np.ascontiguousarray(
            w.T.reshape(i // 128, 128, o).transpose(1, 0, 2)
        ).reshape(128, i // 128 * o)

    pwT = blockT(proj_w).astype(np_bf16)                        # [128, KE*D]
    wgT = np.stack(
        [blockT(w_gate[c] * gamma[c][None, :]) for c in range(C)]
    ).astype(np_bf16)
    wvT = np.stack(
        [blockT(w_val[c] * gamma[c][None, :]) for c in range(C)]
    ).astype(np_bf16)
    woT = np.stack([blockT(w_out[c]) for c in range(C)]).astype(np_bf16)
    bg = np.einsum("cfd,cd->cf", w_gate, beta).astype(np.float32)
    bv = np.einsum("cfd,cd->cf", w_val, beta).astype(np.float32)

    emit_pb = bool(np.any(proj_b))
    emit_gb = bool(np.any(bg) or np.any(bv))

    key = (emit_pb, emit_gb)
    if key not in _BUILD_CACHE:
        _BUILD_CACHE[key] = _build(emit_pb, emit_gb)
    nc = _BUILD_CACHE[key]

    shared = {"pwT": pwT, "wgT": wgT, "wvT": wvT, "woT": woT}
    if emit_pb:
        shared["pb"] = proj_b.reshape(1, D)
    if emit_gb:
        # f-major [C, 128, KD]: bias for f = mf*128 + p lands at [p, mf]
        shared["bg"] = np.ascontiguousarray(
            bg.reshape(C, KD, 128).transpose(0, 2, 1)
        )
        shared["bv"] = np.ascontiguousarray(
            bv.reshape(C, KD, 128).transpose(0, 2, 1)
        )
    in_maps = []
    for i in range(N_CORES):
        m = dict(shared)
        m["th"] = thB[i].reshape(C, NT, 128, KE * TOK)
        in_maps.append(m)

    last_in_maps = in_maps
    trace = bool(int(os.environ.get("KERNEL_TRACE", "0")))
    try:
        last_results = run_bass_kernel_spmd(
            nc, in_maps, core_ids=list(range(N_CORES)), trace=trace
        )
    except ModuleNotFoundError:
        # tracing requested (e.g. BASS_TRACE in env) but the axon NTFF hook
        # module is absent in this image -- rerun without tracing.
        os.environ["BASS_NEVER_TRACE"] = "1"
        last_results = run_bass_kernel_spmd(
            nc, in_maps, core_ids=list(range(N_CORES)), trace=False
        )
    parts = []
    for r in last_results.results:
        o = r["out"].reshape(128, C, NT, G, 36)
        parts.append(o.transpose(1, 2, 3, 0, 4).reshape(C, BLOC, 36))
    full = np.concatenate(parts, axis=1)                        # [C, B, 36]
    return full.reshape(C, B, 6, 6)


# revision 10
# speedup vs baseline: 1.1267x; 1.0125x over previous
"""DeltaMPredictor Trainium2 kernel (8 NeuronCores, data-parallel over batch).

Pipeline per token (b, c):
    reg = thumb @ proj_w.T + proj_b            [2048] -> [512]
    y   = (reg - mean) * rstd                  per-camera LayerNorm (gamma
                                               folded into the SwiGLU weights)
    gate = y @ (w_gate*gamma).T (+ w_gate@beta via ACT bias)
    val  = y @ (w_val *gamma).T (+ w_val @beta via STT bias)
    h   = silu(gate) * val
    A   = reshape(h @ w_out.T, 6, 6); A -= A.T; clip frob to 3
    dM  = expm(A)   even/odd degree-5 Taylor + 1 squaring (4 bprods total):
                    B = As@As, B2 = B@B, E0 = I + B/2 + B2/24 + As@C2,
                    C2 = I + B/6 + B2/120, E = E0@E0

Sharding: batch B=16384 split 8 ways (2048 rows/core); weights replicated.
Per core the loop is camera-major (4 cameras x 4 tiles of 512 tokens).

Perf structure (v2, from trace analysis of the 622us baseline):
  - the 622us baseline lost ~107us to per-tile mm1 stalls waiting on
    thumbnail DMA (tkp bufs=2 releases the buffer too late) plus ~90us of
    HAM clock-gate penalty caused by those >3.4us PE idle gaps, ~43us to
    the expm drain tail, and ~11us to camera-boundary weight-DMA waits.
  - fixes: tkp bufs=3 (DMA issues a full extra tile early); output DMAs
    moved off the sync queue (they head-of-line-blocked thumbnail
    prefetch behind expm completion) onto the gpsimd SWDGE queue; camera
    weights moved from the gpsimd queue to the now-clean sync HWDGE
    queue; expm stage1 runs in the same step as mm3 so the drain tail is
    a single stage2 (~10us, was 3 stages / 43us).
  - engine balance: GPSIMD carries only background expm work (bprod
    multiplies, skew/scale, diag adds, scs pow).  The Taylor combines
    run as fused scalar_tensor_tensor/tensor_scalar ops on DVE.  The LN
    rstd chain moved from GPSIMD pow to DVE reciprocal + ACT sqrt, so no
    foreground op can queue behind a multi-us GPSIMD expm op.
  - y/yT/h and all mm2/mm3 weights are bf16: wg-stationary loads get FWL
    (f32r disables it), the y transposes run 1.0 cyc/row (f32r pays
    1.5x via the identity operand, which prices PE transpose cost), and
    the PSUM->SBUF yT copies run in DVE 2x mode.  Measured warm issue
    rates: bf16 N=512 matmul 216ns, f32r 233ns.
  - a chain of dummy PE transposes warms the HAM clock gate during the
    initial DMA wait, so mm1 starts at 2.4 GHz.
"""

import os
import sys

sys.path.insert(0, "/opt/trn_rl_repo")

from contextlib import ExitStack

import numpy as np

import concourse.bacc as bacc
import concourse.bass as bass
import concourse.tile as tile
from concourse import mybir
from concourse.bass_utils import run_bass_kernel_spmd
from concourse.masks import make_identity

B, C, D_BB, D = 16384, 4, 2048, 512
N_CORES = 8
BLOC = B // N_CORES          # 2048 batch rows per core
TOK = 512                    # tokens per tile
NT = BLOC // TOK             # 4 tiles per camera
G = TOK // 128               # 4 token chunks of 128 per tile
KE = D_BB // 128             # 16 contraction chunks for mm1
KD = D // 128                # 4 contraction chunks for mm2/mm3
MAX_NORM = 3.0
LN_EPS = 1e-5
EXP_S = 1                    # squarings in expm (deg-5 even/odd Taylor)
WARM_N = 48                  # dummy PE transposes to warm the clock gate

F32 = mybir.dt.float32
F32R = mybir.dt.float32r
BF16 = mybir.dt.bfloat16
I32 = mybir.dt.int32
AL = mybir.AluOpType
AF = mybir.ActivationFunctionType
AX = mybir.AxisListType

# Priority boost for the tile-critical chain (LN stats -> rstd -> y -> yT
# copies -> silu -> h).  Priorities only break ties among READY ops, so the
# real protection is the engine split; the boost still helps ties.
FG = 1_000_000

_BUILD_CACHE = {}
last_results = None          # test harness introspection
last_in_maps = None


def _bprod(nc, pool, left, right, tag, out_tile=None, fast=False):
    """out = left @ right per (token, g): one broadcast multiply per g into
    TMP (i, j, l layout), all on GPSIMD (pure background engine), then the
    segmented l-reduce on DVE split in two halves so each foreground-
    blocking op stays ~0.5us.  fast=True (drain tiles only) alternates the
    multiplies over GPSIMD and DVE to halve the serial chain when the other
    engines have gone idle."""
    TMP = pool.tile([128, G * 216], F32, tag="xTMP")
    for g in range(G):
        lv = (
            left[:, g * 36 : (g + 1) * 36]
            .rearrange("p (i l) -> p i l", i=6)
            .unsqueeze(2)
            .broadcast_to((128, 6, 6, 6))          # p i j l
        )
        rv = (
            right[:, g * 36 : (g + 1) * 36]
            .rearrange("p (l j) -> p l j", l=6)
            .unsqueeze(1)
            .broadcast_to((128, 6, 6, 6))          # p i l j
            .transpose([0, 1, 3, 2])               # p i j l
        )
        tmp_v = TMP[:, g * 216 : (g + 1) * 216].rearrange(
            "p (i j l) -> p i j l", i=6, j=6, l=6
        )
        eng = nc.vector if (fast and g % 2 == 1) else nc.gpsimd
        eng.tensor_tensor(out=tmp_v, in0=lv, in1=rv, op=AL.mult)
    if out_tile is None:
        out_tile = pool.tile([128, G * 36], F32, tag=tag)
    hg = G // 2
    for hh in range(2):
        nc.vector.tensor_reduce(
            out=out_tile[:, hh * hg * 36 : (hh + 1) * hg * 36],
            in_=TMP[
                :, hh * hg * 216 : (hh + 1) * hg * 216
            ].rearrange("p (q l) -> p q l", l=6),
            axis=AX.X,
            op=AL.add,
        )
    return out_tile


def _diag_add_one(nc, t, consts):
    # stays on GPSIMD (background): an ACT op here would wait on the GPS/
    # DVE-produced input and head-of-line-block the foreground ACT stream.
    dv = t[:, :].rearrange("p (g a) -> p g a", g=G)[:, :, 0:36:7]
    ones = consts["cone"][:, 0:1].unsqueeze(2).broadcast_to((128, G, 6))
    nc.gpsimd.tensor_tensor(out=dv, in0=dv, in1=ones, op=AL.add)


def _cb(ct, n=G * 36):
    return ct[:, 0:1].broadcast_to((128, n))


def _expm_stage1(nc, pool, A0, consts, fast=False):
    """Skew, frob-clip, B = As@As, B2 = B@B.  Returns tiles for stage 2.

    Elementwise work on GPSIMD via tensor_tensor against broadcast constant
    tiles (NEVER gpsimd.tensor_scalar -- its Q7 software path runs ~1.4us
    per op vs ~0.4-0.7us for tensor_tensor).
    """

    def v4(t):
        return t[:, :].rearrange("p (g i j) -> p g i j", g=G, i=6, j=6)

    S = pool.tile([128, G * 36], F32, tag="xS")
    nc.gpsimd.tensor_tensor(
        out=v4(S), in0=v4(A0), in1=v4(A0).transpose([0, 1, 3, 2]), op=AL.subtract
    )
    SQ = pool.tile([128, G * 36], F32, tag="xSQ")
    nc.gpsimd.tensor_tensor(out=SQ[:, :], in0=S[:, :], in1=S[:, :], op=AL.mult)
    ss = pool.tile([128, G], F32, tag="xss")
    nc.vector.tensor_reduce(
        out=ss[:, :],
        in_=SQ[:, :].rearrange("p (g a) -> p g a", g=G),
        axis=AX.X,
        op=AL.add,
    )
    # scs = min(MAX_NORM/frob, 1)/2^EXP_S, computed clamp-first so the
    # whole chain is DVE -> GPS with no downstream waits and no infinities:
    # scs = pow(max(ss, MN^2) / (MN/2^s)^2, -1/2)
    m2 = pool.tile([128, G], F32, tag="xm2")
    nc.vector.tensor_scalar(
        out=m2[:, :], in0=ss[:, :],
        scalar1=MAX_NORM * MAX_NORM,
        scalar2=(1 << EXP_S) * (1 << EXP_S) / (MAX_NORM * MAX_NORM),
        op0=AL.max, op1=AL.mult,
    )
    scs = pool.tile([128, G], F32, tag="xscs")
    nc.gpsimd.tensor_tensor(
        out=scs[:, :], in0=m2[:, :],
        in1=_cb(consts["neghalf"], G), op=AL.pow,
    )
    As = pool.tile([128, G * 36], F32, tag="xAs")
    nc.gpsimd.tensor_tensor(
        out=As[:, :].rearrange("p (g a) -> p g a", g=G),
        in0=S[:, :].rearrange("p (g a) -> p g a", g=G),
        in1=scs[:, :].unsqueeze(2).broadcast_to((128, G, 36)),
        op=AL.mult,
    )
    Bm = _bprod(nc, pool, As, As, "xB", fast=fast)
    B2 = _bprod(nc, pool, Bm, Bm, "xB2", fast=fast)
    return {"As": As, "Bm": Bm, "B2": B2}


def _expm_stage2(nc, pool, st, out_slice, consts, fast=False):
    """C2 = I + B/6 + B2/120, E0 = I + B/2 + B2/24 + As@C2, E = E0@E0.

    Taylor combines run as fused DVE ops (scalar_tensor_tensor /
    tensor_scalar); only the bprod multiplies and diag adds touch GPSIMD.
    """
    As, Bm, B2 = st["As"], st["Bm"], st["B2"]
    # C2 = (B2/20 + B)/6 + I
    w = pool.tile([128, G * 36], F32, tag="xw2")
    nc.vector.scalar_tensor_tensor(
        out=w[:, :], in0=B2[:, :], scalar=1.0 / 20.0, in1=Bm[:, :],
        op0=AL.mult, op1=AL.add,
    )
    C2 = pool.tile([128, G * 36], F32, tag="xC2")
    nc.vector.tensor_scalar(
        out=C2[:, :], in0=w[:, :], scalar1=1.0 / 6.0, scalar2=None,
        op0=AL.mult,
    )
    _diag_add_one(nc, C2, consts)
    P = _bprod(nc, pool, As, C2, "xP", fast=fast)
    # E0 = (B2/12 + B)/2 + P + I
    E0 = pool.tile([128, G * 36], F32, tag="xE0")
    v = pool.tile([128, G * 36], F32, tag="xv")
    nc.vector.scalar_tensor_tensor(
        out=v[:, :], in0=B2[:, :], scalar=1.0 / 12.0, in1=Bm[:, :],
        op0=AL.mult, op1=AL.add,
    )
    nc.vector.scalar_tensor_tensor(
        out=E0[:, :], in0=v[:, :], scalar=0.5, in1=P[:, :],
        op0=AL.mult, op1=AL.add,
    )
    _diag_add_one(nc, E0, consts)
    _bprod(nc, pool, E0, E0, "xE", out_tile=out_slice, fast=fast)


def _build(emit_pb, emit_gb):
    nc = bacc.Bacc("TRN2", target_bir_lowering=False, debug=False)

    th = nc.dram_tensor("th", [C, NT, 128, KE * TOK], BF16, kind="ExternalInput")
    pwT = nc.dram_tensor("pwT", [128, KE * D], BF16, kind="ExternalInput")
    wgT = nc.dram_tensor("wgT", [C, 128, KD * D], BF16, kind="ExternalInput")
    wvT = nc.dram_tensor("wvT", [C, 128, KD * D], BF16, kind="ExternalInput")
    woT = nc.dram_tensor("woT", [C, 128, KD * 36], BF16, kind="ExternalInput")
    pb = bg = bv = None
    if emit_pb:
        pb = nc.dram_tensor("pb", [1, D], F32R, kind="ExternalInput")
    if emit_gb:
        # per-camera rank-1 LN-beta biases, f-major [128, KD] so column mf is
        # the per-partition bias for f-chunk mf.
        bg = nc.dram_tensor("bg", [C, 128, KD], F32, kind="ExternalInput")
        bv = nc.dram_tensor("bv", [C, 128, KD], F32, kind="ExternalInput")
    out = nc.dram_tensor("out", [128, C * NT * G * 36], F32, kind="ExternalOutput")

    with tile.TileContext(nc) as tc, ExitStack() as ctx:
        singles = ctx.enter_context(tc.tile_pool(name="singles", bufs=1))
        cam = ctx.enter_context(tc.tile_pool(name="cam", bufs=2))
        tkp = ctx.enter_context(tc.tile_pool(name="tkp", bufs=3))
        work = ctx.enter_context(tc.tile_pool(name="work", bufs=2))
        xw = ctx.enter_context(tc.tile_pool(name="xw", bufs=2))
        # PSUM budget (8 banks): transposes+a0 share 2, mm1-reg 3, gate/val 3.
        tr_ps = ctx.enter_context(tc.tile_pool(name="tr_ps", bufs=2, space="PSUM"))
        reg_psp = ctx.enter_context(tc.tile_pool(name="reg_ps", bufs=3, space="PSUM"))
        mm_ps = ctx.enter_context(tc.tile_pool(name="mm_ps", bufs=3, space="PSUM"))

        # memset cannot emit float32r directly; stage in f32 and cast-copy.
        identF = singles.tile([128, 128], F32)
        make_identity(nc, identF)
        identR = singles.tile([128, 128], F32R)
        nc.vector.tensor_copy(identR, identF)
        # PE clock-gate warmup during the initial DMA wait: a WAW chain of
        # transposes whose final result becomes the live transpose identity.
        warm_ps = tr_ps.tile([128, 128], F32R, tag="tr")
        for _ in range(WARM_N):
            nc.tensor.transpose(warm_ps, identR, identR)
        identW = singles.tile([128, 128], F32R)
        nc.scalar.copy(identW, warm_ps)
        # bf16 identity for the y transposes (bf16 transposes run 1 cyc/row;
        # the f32r identity operand would price them at 1.5).
        identB = singles.tile([128, 128], BF16)
        nc.vector.tensor_copy(identB, identF)

        # pw on the scalar HWDGE queue (idle at start) so it streams in
        # parallel with the first thumbnail tile on the sync queue; mm1(t0)
        # is then paced by aggregate HBM bandwidth, not queue serialization.
        pw_s = singles.tile([128, KE, D], BF16)
        pwsrc = pwT.ap().rearrange("p (k d) -> p k d", k=KE)
        for q in range(4):
            nc.scalar.dma_start(
                pw_s[:, q * 4 : (q + 1) * 4, :], pwsrc[:, q * 4 : (q + 1) * 4, :]
            )
        pb_s = None
        ones128 = None
        if emit_pb:
            onesF = singles.tile([1, 128], F32)
            nc.vector.memset(onesF, 1.0)
            ones128 = singles.tile([1, 128], F32R)
            nc.vector.tensor_copy(ones128, onesF)
            pb_s = singles.tile([1, D], F32R)
            nc.gpsimd.dma_start(pb_s, pb.ap())
        consts = {}
        for nm, val in (
            ("neghalf", -0.5),
            ("cone", 1.0),
        ):
            ct = singles.tile([128, 1], F32, name=f"c_{nm}")
            nc.vector.memset(ct, val)
            consts[nm] = ct

        # ---- software pipeline over tiles ------------------------------
        # step tau:  expm-stage2(tau-3)+out-DMA | mm3(tau-2)+expm-stage1 |
        #            mm1+LN(tau) | tr+mm2+silu/h(tau-1)
        # Every PE op in a step depends only on results from a FULL tile
        # earlier, and expm stage1 follows its own mm3 within the step, so
        # the drain after the last PE op is a single stage2 (~10us).

        def emit_mm1_ln(c, ti):
            thsrc = th.ap()[c, ti].rearrange("p (k b) -> p k b", k=KE)
            qk = KE // 4
            thq = []
            for q in range(4):
                t_ = tkp.tile([128, qk, TOK], BF16, tag=f"tk{q}")
                nc.sync.dma_start(t_, thsrc[:, q * qk : (q + 1) * qk, :])
                thq.append(t_)
            y = work.tile([128, G, D], BF16, tag="y", bufs=3)
            mv = work.tile([128, G, 2], F32, tag="mv")
            Ds = [None] * G
            reg_banks = []
            for g in range(G):
                reg_ps = reg_psp.tile([128, D], F32, tag="reg")
                reg_banks.append(reg_ps)
                for k in range(KE):
                    nc.tensor.matmul(
                        reg_ps,
                        thq[k // qk][:, k % qk, g * 128 : (g + 1) * 128],
                        pw_s[:, k, :],
                        start=(k == 0),
                        stop=(k == KE - 1 and not emit_pb),
                    )
                if emit_pb:
                    nc.tensor.matmul(
                        reg_ps, ones128, pb_s, start=False, stop=True
                    )
                # DVE stats; rstd via DVE reciprocal + ACT sqrt (no GPSIMD
                # in the foreground chain); ACT y pair.
                with tc.high_priority(offset=FG):
                    st = work.tile([128, 6], F32, tag="bst")
                    nc.vector.bn_stats(out=st[:, :], in_=reg_ps)
                    nc.vector.bn_aggr(out=mv[:, g, :], in_=st[:, :])
                    if g % 2 == 1:
                        p0 = g - 1
                        varv = mv[:, p0 : g + 1, 1:2].rearrange(
                            "p a b -> p (a b)"
                        )
                        meanv = mv[:, p0 : g + 1, 0:1].rearrange(
                            "p a b -> p (a b)"
                        )
                        iv = work.tile([128, 2], F32, tag="lniv")
                        nc.vector.tensor_scalar(
                            out=iv[:, :], in0=varv, scalar1=LN_EPS,
                            scalar2=None, op0=AL.add,
                        )
                        rv = work.tile([128, 2], F32, tag="lnrv")
                        nc.vector.reciprocal(rv[:, :], iv[:, :])
                        rstd = work.tile([128, 2], F32, tag="lnrs")
                        nc.scalar.sqrt(rstd[:, :], rv[:, :])
                        nmr = work.tile([128, 2], F32, tag="lnnm")
                        nc.vector.scalar_tensor_tensor(
                            out=nmr[:, :], in0=meanv, scalar=-1.0,
                            in1=rstd[:, :], op0=AL.mult, op1=AL.mult,
                        )
                        for gg in (p0, g):
                            nc.scalar.activation(
                                y[:, gg, :], reg_banks[gg], AF.Identity,
                                bias=nmr[:, gg - p0 : gg - p0 + 1],
                                scale=rstd[:, gg - p0 : gg - p0 + 1],
                            )
                            Ds[gg] = identB
            return y, Ds

        def emit_tr_mm2(sA):
            y = sA["y"]
            yT = work.tile([128, KD, TOK], BF16, tag="yT", bufs=1)
            for kd in range(KD):
                tr = tr_ps.tile([128, TOK], BF16, tag="tr")
                for g in range(G):
                    nc.tensor.transpose(
                        tr[:, g * 128 : (g + 1) * 128],
                        y[:, g, kd * 128 : (kd + 1) * 128],
                        sA["D"][g],
                    )
                with tc.high_priority(offset=FG):
                    if kd % 2 == 0:
                        nc.scalar.copy(yT[:, kd, :], tr)
                    else:
                        nc.vector.tensor_copy(yT[:, kd, :], tr)
            h = work.tile([128, KD, TOK], BF16, tag="h")
            for mf in range(KD):
                g_ps = mm_ps.tile([128, TOK], F32, tag="mm")
                for kd in range(KD):
                    nc.tensor.matmul(
                        g_ps,
                        sA["wg"][:, kd, mf * 128 : (mf + 1) * 128],
                        yT[:, kd, :],
                        start=(kd == 0),
                        stop=(kd == KD - 1),
                    )
                v_ps = mm_ps.tile([128, TOK], F32, tag="mm")
                for kd in range(KD):
                    nc.tensor.matmul(
                        v_ps,
                        sA["wv"][:, kd, mf * 128 : (mf + 1) * 128],
                        yT[:, kd, :],
                        start=(kd == 0),
                        stop=(kd == KD - 1),
                    )
                sg = work.tile([128, TOK], F32, tag="sg", bufs=2)
                with tc.high_priority(offset=FG):
                    if emit_gb:
                        nc.scalar.activation(
                            sg, g_ps, AF.Silu, bias=sA["bg"][:, mf : mf + 1]
                        )
                        nc.vector.scalar_tensor_tensor(
                            out=h[:, mf, :], in0=v_ps,
                            scalar=sA["bv"][:, mf : mf + 1], in1=sg,
                            op0=AL.add, op1=AL.mult,
                        )
                    else:
                        nc.scalar.activation(sg, g_ps, AF.Silu)
                        nc.vector.tensor_tensor(
                            out=h[:, mf, :], in0=sg, in1=v_ps, op=AL.mult
                        )
            return h

        def emit_mm3(sB):
            a0_ps = tr_ps.tile([128, G, 36], F32R, tag="tr")
            a0T_ps = mm_ps.tile([36, TOK], F32, tag="mm", name="a0T")
            for kf in range(KD):
                nc.tensor.matmul(
                    a0T_ps,
                    sB["wo"][:, kf, :],
                    sB["h"][:, kf, :],
                    start=(kf == 0),
                    stop=(kf == KD - 1),
                )
            a0T_s = work.tile([36, TOK], F32R, tag="a0T")
            with tc.high_priority(offset=FG):
                nc.scalar.copy(a0T_s, a0T_ps)
            for g in range(G):
                nc.tensor.transpose(
                    a0_ps[:, g, :],
                    a0T_s[:, g * 128 : (g + 1) * 128],
                    identW[:36, :36],
                )
            A0 = xw.tile([128, G * 36], F32, tag="A0", bufs=3)
            with tc.high_priority(offset=FG):
                nc.scalar.copy(
                    A0, a0_ps[:, :, :].rearrange("p g a -> p (g a)")
                )
            return A0

        sA1 = sA2 = None     # tile tau-1 / tau-2 pipeline state
        q2 = None            # expm awaiting stage 2:  (state, idx)

        tiles = [(c, ti) for c in range(C) for ti in range(NT)]
        n_tiles = len(tiles)
        wg_s = wv_s = wo_s = bg_s = bv_s = None
        for step in range(n_tiles + 3):
            # stage 2 of the tile whose stage 1 ran last step; the output
            # DMA rides the gpsimd SWDGE queue (the sync queue must stay
            # free for thumbnail prefetch).
            pending = None
            if q2 is not None:
                st2, pidx = q2
                Eout = xw.tile([128, G * 36], F32, tag="Eout")
                _expm_stage2(nc, xw, st2, Eout, consts, fast=(pidx >= 13))
                pending = (pidx, Eout)
                q2 = None
            # transposes + mm2 consume y from TWO tiles back: the LN chain
            # (stats -> aggr -> recip -> sqrt -> nmr -> y) takes ~9us after
            # mm1's last matmul, so with a 1-step lag the PE stalled ~4us
            # per tile waiting for y.  mm3 + expm stage 1 run on the fresh
            # h in the same step, keeping the drain a single stage 2.
            if sA2 is not None:
                h = emit_tr_mm2(sA2)
                A0 = emit_mm3({"h": h, "wo": sA2["wo"]})
                fast = sA2["idx"] >= 13
                q2 = (_expm_stage1(nc, xw, A0, consts, fast=fast), sA2["idx"])
            if pending is not None:
                pidx, Eout = pending
                nc.gpsimd.dma_start(
                    out.ap()[:, pidx * G * 36 : (pidx + 1) * G * 36], Eout
                )
            if step < n_tiles:
                c, ti = tiles[step]
                y, Ds = emit_mm1_ln(c, ti)
                if ti == 0:
                    # camera weights on the sync HWDGE queue, emitted after
                    # this tile's thumbnail DMAs: they are requested ~6
                    # tiles before first use (cam bufs=2), so they never
                    # delay a thumbnail DMA behind them.
                    wg_s = cam.tile([128, KD, D], BF16, tag="wg")
                    nc.sync.dma_start(
                        wg_s, wgT.ap()[c].rearrange("p (k f) -> p k f", k=KD)
                    )
                    wv_s = cam.tile([128, KD, D], BF16, tag="wv")
                    nc.sync.dma_start(
                        wv_s, wvT.ap()[c].rearrange("p (k f) -> p k f", k=KD)
                    )
                    wo_s = cam.tile([128, KD, 36], BF16, tag="wo")
                    nc.sync.dma_start(
                        wo_s, woT.ap()[c].rearrange("p (k o) -> p k o", k=KD)
                    )
                    if emit_gb:
                        bg_s = cam.tile([128, KD], F32, tag="bg")
                        nc.sync.dma_start(bg_s, bg.ap()[c])
                        bv_s = cam.tile([128, KD], F32, tag="bv")
                        nc.sync.dma_start(bv_s, bv.ap()[c])
            sA2 = sA1
            if step < n_tiles:
                sA1 = {
                    "y": y, "D": Ds, "wg": wg_s, "wv": wv_s, "wo": wo_s,
                    "bg": bg_s, "bv": bv_s, "idx": c * NT + ti,
                }
            else:
                sA1 = None

    nc.compile()
    return nc


def kernel(**inputs):
    global last_results, last_in_maps
    np_bf16 = mybir.dt.np(BF16)
    thumb = np.asarray(inputs["thumbnails"], dtype=np.float32)
    # [B, C, E] -> per-core [C, NT, 128, KE, TOK] so each tile lands with one
    # DMA whose per-partition data is contiguous (128 descriptors, not 2048).
    # bf16 halves the DMA bytes that pace both startup and the steady-state
    # thumbnail stream; matmul speed is unchanged (1 cyc/row either way).
    thB = thumb.reshape(N_CORES, NT, TOK, C, KE, 128)
    thB = np.ascontiguousarray(thB.transpose(0, 3, 1, 5, 4, 2)).astype(np_bf16)
    proj_w = np.asarray(inputs["proj_w"], dtype=np.float32)
    proj_b = np.asarray(inputs["proj_b"], dtype=np.float32)
    gamma = np.asarray(inputs["gamma"], dtype=np.float32)
    beta = np.asarray(inputs["beta"], dtype=np.float32)
    w_gate = np.asarray(inputs["w_gate"], dtype=np.float32)
    w_val = np.asarray(inputs["w_val"], dtype=np.float32)
    w_out = np.asarray(inputs["w_out"], dtype=np.float32)

    # host-side weight prep: fold gamma into the SwiGLU weights, beta into
    # rank-1 biases, pre-transpose everything for the PE's lhsT convention.
    def blockT(w):
        # [out, in] -> in-major [128, k, out]: lhsT chunks contiguous/partition
        o, i = w.shape
        return np.ascontiguousarray(
            w.T.reshape(i // 128, 128, o).transpose(1, 0, 2)
        ).reshape(128, i // 128 * o)

    pwT = blockT(proj_w).astype(np_bf16)                        # [128, KE*D]
    wgT = np.stack(
        [blockT(w_gate[c] * gamma[c][None, :]) for c in range(C)]
    ).astype(np_bf16)
    wvT = np.stack(
        [blockT(w_val[c] * gamma[c][None, :]) for c in range(C)]
    ).astype(np_bf16)
    woT = np.stack([blockT(w_out[c]) for c in range(C)]).astype(np_bf16)
    bg = np.einsum("cfd,cd->cf", w_gate, beta).astype(np.float32)
    bv = np.einsum("cfd,cd->cf", w_val, beta).astype(np.float32)

    emit_pb = bool(np.any(proj_b))
    emit_gb = bool(np.any(bg) or np.any(bv))

    key = (emit_pb, emit_gb)
    if key not in _BUILD_CACHE:
        _BUILD_CACHE[key] = _build(emit_pb, emit_gb)
    nc = _BUILD_CACHE[key]

    shared = {"pwT": pwT, "wgT": wgT, "wvT": wvT, "woT": woT}
    if emit_pb:
        shared["pb"] = proj_b.reshape(1, D)
    if emit_gb:
        # f-major [C, 128, KD]: bias for f = mf*128 + p lands at [p, mf]
        shared["bg"] = np.ascontiguousarray(
            bg.reshape(C, KD, 128).transpose(0, 2, 1)
        )
        shared["bv"] = np.ascontiguousarray(
            bv.reshape(C, KD, 128).transpose(0, 2, 1)
        )
    in_maps = []
    for i in range(N_CORES):
        m = dict(shared)
        m["th"] = thB[i].reshape(C, NT, 128, KE * TOK)
        in_maps.append(m)

    last_in_maps = in_maps
    trace = bool(int(os.environ.get("KERNEL_TRACE", "0")))
    try:
        last_results = run_bass_kernel_spmd(
            nc, in_maps, core_ids=list(range(N_CORES)), trace=trace
        )
    except ModuleNotFoundError:
        # tracing requested (e.g. BASS_TRACE in env) but the axon NTFF hook
        # module is absent in this image -- rerun without tracing.
        os.environ["BASS_NEVER_TRACE"] = "1"
        last_results = run_bass_kernel_spmd(
            nc, in_maps, core_ids=list(range(N_CORES)), trace=False
        )
    parts = []
    for r in last_results.results:
        o = r["out"].reshape(128, C, NT, G, 36)
        parts.append(o.transpose(1, 2, 3, 0, 4).reshape(C, BLOC, 36))
    full = np.concatenate(parts, axis=1)                        # [C, B, 36]
    return full.reshape(C, B, 6, 6)
